# revision 70
# baseline (speedup 1.0000x reference)
"""Multi-head attention (B=2, N=2048, D=1024, 16 heads x 64) on 8 NeuronCores.

Sharding: data-parallel over batch (2) x tensor-parallel over heads (4 heads
per core). Each core computes q/k/v projections + RoPE + attention for its 4
heads and a partial output projection; the host sums the 4 tensor-parallel
partials per batch and adds the output bias.

Device kernel notes:
 - x / rope cos/sin are shipped and held in SBUF as bf16 (DMA is a single
   ~350 GB/s stream and otherwise gates the start); weights and all
   intermediates stay float32r (full-rate on the TRN2 PE).
 - One long PE stream: only the dc0 half of K (all chunks) plus the dc0
   half of Q chunk 0 are projected up front, then attention head 0 starts
   (~20us in) and every remaining projection / V-tile / output-projection
   piece is hand-placed as PE filler between attention score/ctx groups,
   sized so the exp (ACT) engine -- the attention-phase co-bottleneck --
   is never waited on.  In-attention projections are emitted as two
   4-matmul parcels accumulating in a dedicated PSUM bank so filler
   granularity matches the per-head ACT deficit.
 - RoPE is applied on channel-permuted q/k (evens-then-odds per 64-channel
   head block, folded into the weight slices host-side) so the rotate-pair
   step is a 32-partition-block swap: a permutation matmul pre-attention,
   and SBUF->SBUF DMAs (4 partition-block copies) for in-attention
   projections where PSUM banks and PE rows are the scarcer resource.
 - Scores are computed transposed (S^T[k, q]) so exp output feeds the ctx
   matmul directly; the softmax denominator comes from a ones-column
   appended to V; no per-row max -- a constant shift of 20 (free via the
   ACT bias, cancels in the ratio) keeps exp in fp32 range.
 - Softmax normalize: reciprocal on the denominator row (partition 64),
   a gpsimd cross-partition copy to partition 0 (HW partition_broadcast
   only reads partition 0), gpsimd broadcast, then one DVE multiply --
   no DMA in the chain.
 - The last q-chunk runs as two 256-column half-chunks (exp groups of 4
   k-tiles keep the ACT tiles 1024 wide) so the final oproj pieces of the
   first half overlap the second half and the drain is short.
"""
import sys

sys.path.insert(0, "/opt/trn_rl_repo")

import numpy as np
import ml_dtypes

import concourse.bacc as bacc
import concourse.mybir as mybir
import concourse.tile as tile
from concourse import bass_utils

B, N, D = 2, 2048, 1024
HEADS, HD = 16, 64
TP = 4                 # tensor-parallel ways (heads)
DP = 2                 # data-parallel ways (batch)
HPC = HEADS // TP      # heads per core = 4
C = HPC * HD           # channels per core = 256
CH = 512               # n-chunk size
NCH = N // CH          # 4
KT = 128               # k tile
NKT = N // KT          # 16
VW = HD + 1            # V columns per head incl. ones column = 65
ITC = D // KT          # 8 contraction tiles for projections
F32R = mybir.dt.float32r
F32 = mybir.dt.float32
BF16 = mybir.dt.bfloat16

_CACHE = {}


def _build():
    nc = bacc.Bacc("TRN2", debug=False, num_devices=DP * TP)

    xT = nc.dram_tensor("xT", [D, N], BF16, kind="ExternalInput").ap()
    cosT = nc.dram_tensor("cosT", [C, N], BF16, kind="ExternalInput").ap()
    sinT = nc.dram_tensor("sinT", [C, N], BF16, kind="ExternalInput").ap()
    wq = nc.dram_tensor("wq", [D, C], BF16, kind="ExternalInput").ap()
    wk = nc.dram_tensor("wk", [D, C], BF16, kind="ExternalInput").ap()
    wvx = nc.dram_tensor("wvx", [D, HPC * VW], BF16, kind="ExternalInput").ap()
    bvb = nc.dram_tensor("bvb", [128, HPC * VW], F32, kind="ExternalInput").ap()
    bqk = nc.dram_tensor("bqk", [2, 2, 128], F32, kind="ExternalInput").ap()
    woT = nc.dram_tensor("woT", [C, D], F32R, kind="ExternalInput").ap()
    eyesw = nc.dram_tensor("eyesw", [128, 128], F32R, kind="ExternalInput").ap()
    ones64 = nc.dram_tensor("ones64", [1, 64], F32R, kind="ExternalInput").ap()
    # partial sums only (host adds the 4 TP partials + bias) -- bf16 halves
    # the output DMA and shortens the drain
    out = nc.dram_tensor("out", [N, D], BF16, kind="ExternalOutput").ap()

    with tile.TileContext(nc) as tc:
        with tc.tile_pool(name="pers", bufs=1) as pers, \
             tc.tile_pool(name="wrk", bufs=1) as wrk, \
             tc.tile_pool(name="psp", bufs=1, space="PSUM") as psp:
            # warmup seed needs no DMA: PE can start its streak immediately
            wseed = pers.tile([128, 128], BF16, tag="wseed")
            nc.gpsimd.memset(wseed[:], 0.0)
            ones64_sb = pers.tile([128, 64], F32R, tag="ones64")
            shift_sb = pers.tile([128, 1], F32, tag="shift")
            nc.gpsimd.memset(shift_sb[:], -20.0)
            # tiny dummy exp: pulls the ACT exp-table load (1.3us) to t~1us
            dume = wrk.tile([1, 8], F32, tag="dume")
            nc.gpsimd.memset(dume[:], 0.5)
            dume2 = wrk.tile([1, 8], F32, tag="dume2")
            nc.scalar.activation(dume2[:], dume[:],
                                 mybir.ActivationFunctionType.Exp)

            # ---- persistent SBUF; DMA emission order = arrival order on the
            # single ~350GB/s stream, so it is chosen to match first use ----
            eye_sb = pers.tile([128, 128], F32R, tag="eyesw")
            nc.sync.dma_start(eye_sb[:], eyesw)
            nc.sync.dma_start(ones64_sb[HD:HD + 1, :], ones64)
            wk_sb = pers.tile([128, ITC, C], BF16, tag="wk")
            wq_sb = pers.tile([128, ITC, C], BF16, tag="wq")
            xt_sb = pers.tile([128, NCH, ITC, CH], BF16, tag="xt")
            cos_sb = pers.tile([128, 2, NCH, CH], BF16, tag="cos")
            sin_sb = pers.tile([128, 2, NCH, CH], BF16, tag="sin")
            wv_sb = pers.tile([128, ITC, HPC * VW], BF16, tag="wv")
            bvb_sb = pers.tile([128, HPC * VW], F32, tag="bvb")
            bqk_sb = pers.tile([128, 2, 2], F32, tag="bqk")
            wo_sb = pers.tile([128, 2, D], F32R, tag="wo")

            def load_w_dc(w_sb, w, dc):
                nc.sync.dma_start(
                    w_sb[:, :, 128 * dc:128 * (dc + 1)],
                    w[:, 128 * dc:128 * (dc + 1)].rearrange(
                        "(t p) c -> p t c", p=128))

            def load_x(c):
                half = ITC // 2
                nc.sync.dma_start(
                    xt_sb[:, c, :half, :],
                    xT[:half * 128, c * CH:(c + 1) * CH].rearrange(
                        "(t p) n -> p t n", p=128))
                nc.sync.dma_start(
                    xt_sb[:, c, half:, :],
                    xT[half * 128:, c * CH:(c + 1) * CH].rearrange(
                        "(t p) n -> p t n", p=128))

            def load_cs(c, t):
                nc.sync.dma_start(
                    cos_sb[:, t, c, :],
                    cosT[128 * t:128 * (t + 1), c * CH:(c + 1) * CH])
                nc.sync.dma_start(
                    sin_sb[:, t, c, :],
                    sinT[128 * t:128 * (t + 1), c * CH:(c + 1) * CH])

            load_w_dc(wk_sb, wk, 0)
            load_x(0)
            load_cs(0, 0)
            nc.sync.dma_start(bqk_sb[:], bqk.rearrange("a c p -> p a c"))
            load_x(1)
            load_cs(1, 0)
            load_w_dc(wq_sb, wq, 0)
            load_x(2)
            load_cs(2, 0)
            load_x(3)
            load_cs(3, 0)
            nc.sync.dma_start(
                wv_sb[:], wvx.rearrange("(t p) c -> p t c", p=128))
            nc.sync.dma_start(bvb_sb[:], bvb)
            load_w_dc(wk_sb, wk, 1)
            load_w_dc(wq_sb, wq, 1)
            for c in range(NCH):
                load_cs(c, 1)
            nc.sync.dma_start(wo_sb[:], woT.rearrange("(t p) o -> p t o", p=128))

            # p-state warmup: keep the PE streak alive through the initial
            # DMA wait so the first projections run at full clock
            wps = psp.tile([128, 128], F32, tag="aux", bufs=1, name="warmup")
            for wi in range(20):
                nc.tensor.matmul(wps[:], lhsT=wseed[:], rhs=wseed[:],
                                 start=True, stop=True)
            for wi in range(18):
                nc.tensor.matmul(wps[:], lhsT=eye_sb[:], rhs=eye_sb[:],
                                 start=True, stop=True)
            qrot = [pers.tile([128, N], F32R, tag=f"qrot{t}", name=f"qrot{t}")
                    for t in range(2)]
            krot = [pers.tile([128, N], F32R, tag=f"krot{t}", name=f"krot{t}")
                    for t in range(2)]
            v_sb = [pers.tile([128, HPC * VW], F32R, tag=f"v{t}", name=f"v{t}")
                    for t in range(NKT)]
            ctxT = [pers.tile([128, N], F32R, tag=f"ctxT{t}", name=f"ctxT{t}")
                    for t in range(2)]

            # pre-attention projections in two stages so the rotate eye
            # matmul (gated by a DVE add) never head-of-line blocks the PE
            _pa = {}

            def rope_a_mm(w_sb, qk, key, c, dc):
                # scores psum (tag "st") is idle pre-attention -- use its
                # two banks as the proj/rotate pair
                stt = psp.tile([128, 2 * CH], F32, tag="st", bufs=2)
                ps, pssh = stt[:, 0:CH], stt[:, CH:2 * CH]
                for it in range(ITC):
                    nc.tensor.matmul(
                        ps, lhsT=w_sb[:, it, 128 * dc:128 * (dc + 1)],
                        rhs=xt_sb[:, c, it, :],
                        start=(it == 0), stop=(it == ITC - 1))
                raw = wrk.tile([128, CH], F32R, tag="raw", bufs=4)
                nc.vector.tensor_scalar_add(
                    raw[:], ps, bqk_sb[:, qk, dc:dc + 1])
                _pa[key] = (raw, pssh)

            def rope_a_fin(key, dst, c, dc):
                raw, pssh = _pa.pop(key)
                nc.tensor.matmul(pssh, lhsT=eye_sb[:], rhs=raw[:],
                                 start=True, stop=True)
                m1 = wrk.tile([128, CH], F32, tag="m1", bufs=2)
                nc.vector.tensor_mul(m1[:], raw[:], cos_sb[:, dc, c, :])
                m2 = wrk.tile([128, CH], F32, tag="m2", bufs=2)
                nc.vector.tensor_mul(m2[:], pssh, sin_sb[:, dc, c, :])
                nc.vector.tensor_add(
                    dst[dc][:, c * CH:(c + 1) * CH], m1[:], m2[:])

            # in-attention projections: two 4-matmul parcels into the
            # dedicated "qp" bank; rotate-swap via 4 SBUF->SBUF DMAs
            _pb = {}

            def proj_b_A(w_sb, dc, c):
                ps = psp.tile([128, CH], F32, tag="qp", bufs=1)
                for it in range(4):
                    nc.tensor.matmul(
                        ps[:], lhsT=w_sb[:, it, 128 * dc:128 * (dc + 1)],
                        rhs=xt_sb[:, c, it, :],
                        start=(it == 0), stop=False)
                _pb["ps"] = ps

            def proj_b_B(w_sb, qk, dst, c, dc):
                ps = _pb.pop("ps")
                for it in range(4, ITC):
                    nc.tensor.matmul(
                        ps[:], lhsT=w_sb[:, it, 128 * dc:128 * (dc + 1)],
                        rhs=xt_sb[:, c, it, :],
                        start=False, stop=(it == ITC - 1))
                raw = wrk.tile([128, CH], F32R, tag="raw", bufs=4)
                nc.vector.tensor_scalar_add(
                    raw[:], ps[:], bqk_sb[:, qk, dc:dc + 1])
                rswp = wrk.tile([128, CH], F32R, tag="rswp", bufs=2)
                for b in range(4):
                    nc.sync.dma_start(rswp[32 * b:32 * (b + 1), :],
                                      raw[32 * (b ^ 1):32 * ((b ^ 1) + 1), :])
                m1 = wrk.tile([128, CH], F32, tag="m1", bufs=2)
                nc.vector.tensor_mul(m1[:], raw[:], cos_sb[:, dc, c, :])
                m2 = wrk.tile([128, CH], F32, tag="m2", bufs=2)
                nc.vector.tensor_mul(m2[:], rswp[:], sin_sb[:, dc, c, :])
                nc.vector.tensor_add(
                    dst[dc][:, c * CH:(c + 1) * CH], m1[:], m2[:])

            def proj_b_B_eye(w_sb, qk, dst, c, dc):
                ps = _pb.pop("ps")
                for it in range(4, ITC):
                    nc.tensor.matmul(
                        ps[:], lhsT=w_sb[:, it, 128 * dc:128 * (dc + 1)],
                        rhs=xt_sb[:, c, it, :],
                        start=False, stop=(it == ITC - 1))
                raw = wrk.tile([128, CH], F32R, tag="raw", bufs=4)
                nc.vector.tensor_scalar_add(
                    raw[:], ps[:], bqk_sb[:, qk, dc:dc + 1])
                pssh = psp.tile([128, CH], F32, tag="aux", bufs=1)
                nc.tensor.matmul(pssh[:], lhsT=eye_sb[:], rhs=raw[:],
                                 start=True, stop=True)
                m1 = wrk.tile([128, CH], F32, tag="m1", bufs=2)
                nc.vector.tensor_mul(m1[:], raw[:], cos_sb[:, dc, c, :])
                m2 = wrk.tile([128, CH], F32, tag="m2", bufs=2)
                nc.vector.tensor_mul(m2[:], pssh[:], sin_sb[:, dc, c, :])
                nc.vector.tensor_add(
                    dst[dc][:, c * CH:(c + 1) * CH], m1[:], m2[:])

            def vproj_tile(c, vt):
                kt = c * (CH // KT) + vt
                psv = psp.tile([128, CH], F32, tag="aux", bufs=1)
                for it in range(ITC):
                    nc.tensor.matmul(
                        psv[:, 0:HPC * VW],
                        lhsT=xt_sb[:, c, it, KT * vt:KT * (vt + 1)],
                        rhs=wv_sb[:, it, :],
                        start=(it == 0), stop=(it == ITC - 1))
                nc.vector.tensor_add(v_sb[kt][:], psv[:, 0:HPC * VW], bvb_sb[:])

            # one bf16 staging tile per KT-row piece; both oc halves copy in,
            # the second flushes with a single DMA
            _obs = {}

            def oproj_half(qc8, nt, oc, pstag="aux"):
                n0 = qc8 * CH + nt * KT
                pso = psp.tile([128, 2 * CH] if pstag == "st" else [128, CH],
                               F32, tag=pstag, bufs=2 if pstag == "st" else 1)
                for it in range(2):
                    nc.tensor.matmul(
                        pso[:, 0:CH], lhsT=ctxT[it][:, n0:n0 + KT],
                        rhs=wo_sb[:, it, CH * oc:CH * (oc + 1)],
                        start=(it == 0), stop=(it == 1))
                if oc == 0:
                    ob = wrk.tile([128, D], BF16, tag="ob", bufs=4)
                    _obs[(qc8, nt)] = ob
                else:
                    ob = _obs.pop((qc8, nt))
                nc.vector.tensor_copy(ob[:, CH * oc:CH * (oc + 1)],
                                      pso[:, 0:CH])
                if oc == 1:
                    nc.sync.dma_start(out[n0:n0 + KT, :], ob[:])

            # ---- filler units ----
            def U_kA(c):
                return lambda: proj_b_A(wk_sb, 1, c)

            def U_kB(c):
                return lambda: proj_b_B(wk_sb, 1, krot, c, 1)

            def U_qA(c, dc):
                return lambda: proj_b_A(wq_sb, dc, c)

            def U_qB(c, dc):
                return lambda: proj_b_B(wq_sb, 0, qrot, c, dc)

            def U_k0A(c):
                return lambda: proj_b_A(wk_sb, 0, c)

            def U_k0B(c):
                return lambda: proj_b_B_eye(wk_sb, 1, krot, c, 0)

            def U_v(c, vt):
                return lambda: vproj_tile(c, vt)

            def U_op(qc8, nt, oc):
                return lambda: oproj_half(qc8, nt, oc)

            _opq = {}

            def U_opA(qc8, nt, oc):
                def f():
                    n0 = qc8 * CH + nt * KT
                    pso = psp.tile([128, CH], F32, tag="aux", bufs=1,
                                   name=f"opq{qc8}_{nt}_{oc}")
                    nc.tensor.matmul(
                        pso[:], lhsT=ctxT[0][:, n0:n0 + KT],
                        rhs=wo_sb[:, 0, CH * oc:CH * (oc + 1)],
                        start=True, stop=False)
                    _opq[(qc8, nt, oc)] = pso
                return f

            def U_opB(qc8, nt, oc):
                def f():
                    n0 = qc8 * CH + nt * KT
                    pso = _opq.pop((qc8, nt, oc))
                    nc.tensor.matmul(
                        pso[:], lhsT=ctxT[1][:, n0:n0 + KT],
                        rhs=wo_sb[:, 1, CH * oc:CH * (oc + 1)],
                        start=False, stop=True)
                    if oc == 0:
                        ob = wrk.tile([128, D], BF16, tag="ob", bufs=4)
                        _obs[(qc8, nt)] = ob
                    else:
                        ob = _obs.pop((qc8, nt))
                    nc.vector.tensor_copy(ob[:, CH * oc:CH * (oc + 1)],
                                          pso[:])
                    if oc == 1:
                        nc.sync.dma_start(out[n0:n0 + KT, :], ob[:])
                return f

            def attention(q0, qw, gk, horder, fill, pe_bcast_last=False):
                """Attention for q columns [q0, q0+qw); gk k-tiles per exp
                group. fill: {(hi, g): [units]} emitted after scores(g);
                g == ngr means after the head's last ctx."""
                ngr = NKT // gk
                qs = slice(q0, q0 + qw)
                for hi, h in enumerate(horder):
                    pt, par = h // 2, h % 2
                    cx = psp.tile([128, CH], F32, tag="cx", bufs=2)
                    r0 = 64 * par
                    es_q = [None] * ngr

                    def scores(g):
                        stg = psp.tile([128, 2 * CH], F32, tag="st",
                                       bufs=2, name=f"st{q0}_{h}_{g}")
                        for j in range(gk):
                            kt = gk * g + j
                            nc.tensor.matmul(
                                stg[:, qw * j:qw * (j + 1)],
                                lhsT=krot[pt][r0:r0 + 64, KT * kt:KT * (kt + 1)],
                                rhs=qrot[pt][r0:r0 + 64, qs],
                                start=True, stop=True)
                        es = wrk.tile([128, 2 * CH], F32R, tag="es",
                                      bufs=4, name=f"es{q0}_{h}_{g}")
                        # constant shift cancels in the softmax ratio but
                        # widens the no-max-subtraction overflow envelope
                        nc.scalar.activation(
                            es[:, 0:gk * qw], stg[:, 0:gk * qw],
                            mybir.ActivationFunctionType.Exp,
                            bias=shift_sb[:])
                        es_q[g] = es

                    def ctx_acc(g):
                        for j in range(gk):
                            kt = gk * g + j
                            nc.tensor.matmul(
                                cx[0:VW, 0:qw],
                                lhsT=v_sb[kt][:, VW * h:VW * (h + 1)],
                                rhs=es_q[g][:, qw * j:qw * (j + 1)],
                                start=(kt == 0), stop=(kt == NKT - 1))

                    def emit_fill(g):
                        for u in fill.get((hi, g), ()):
                            u()

                    scores(0)
                    emit_fill(0)
                    for g in range(1, ngr):
                        scores(g)
                        emit_fill(g)
                        ctx_acc(g - 1)
                    ctx_acc(ngr - 1)
                    emit_fill(ngr)

                    rt = wrk.tile([128, CH], F32, tag="rt", bufs=2)
                    nc.vector.reciprocal(rt[HD:HD + 1, 0:qw],
                                         cx[HD:HD + 1, 0:qw])
                    # hop the denominator row to partition 0 with a
                    # gpsimd copy (HW partition_broadcast only reads
                    # partition 0; cross-partition gpsimd access works for
                    # 32-aligned partitions), then broadcast -- no DMA
                    rt0 = wrk.tile([1, CH], F32, tag="rt0", bufs=2)
                    nc.gpsimd.tensor_copy(rt0[:, 0:qw], rt[HD:HD + 1, 0:qw])
                    rb = wrk.tile([64, CH], F32, tag="rb", bufs=2)
                    nc.gpsimd.partition_broadcast(rb[:, 0:qw],
                                                  rt0[:, 0:qw])
                    rbv = rb
                    if par == 0:
                        if pe_bcast_last and hi == len(horder) - 1:
                            # split by column half: each final oproj piece
                            # unblocks as soon as its columns are scaled
                            hw_ = qw // 2
                            nc.vector.tensor_mul(
                                ctxT[pt][0:64, q0:q0 + hw_],
                                cx[0:64, 0:hw_], rbv[:, 0:hw_])
                            nc.vector.tensor_mul(
                                ctxT[pt][0:64, q0 + hw_:q0 + qw],
                                cx[0:64, hw_:qw], rbv[:, hw_:qw])
                        else:
                            nc.vector.tensor_mul(
                                ctxT[pt][0:64, qs], cx[0:64, 0:qw],
                                rbv[:, 0:qw])
                    else:
                        ch_t = wrk.tile([64, CH], F32R, tag="ch", bufs=2)
                        nc.vector.tensor_mul(ch_t[:, 0:qw], cx[0:64, 0:qw],
                                             rbv[:, 0:qw])
                        nc.sync.dma_start(ctxT[pt][64:128, qs], ch_t[:, 0:qw])

            # ======== pre-attention: pure K dc0 + Q0 dc0 chain -- x3 is
            # the critical-path entry, so V0/V1 ride inside attention head 0
            # and all x chunks precede wv on the DMA stream
            rope_a_mm(wk_sb, 1, "k0", 0, 0)
            rope_a_mm(wk_sb, 1, "k1", 1, 0)
            rope_a_fin("k0", krot, 0, 0)
            rope_a_mm(wq_sb, 0, "q0", 0, 0)
            rope_a_fin("k1", krot, 1, 0)
            rope_a_mm(wk_sb, 1, "k2", 2, 0)
            rope_a_fin("q0", qrot, 0, 0)
            for wi in range(8):
                nc.tensor.matmul(wps[:], lhsT=eye_sb[:], rhs=eye_sb[:],
                                 start=True, stop=True)
            rope_a_mm(wk_sb, 1, "k3", 3, 0)
            rope_a_fin("k2", krot, 2, 0)
            rope_a_fin("k3", krot, 3, 0)

            # ======== attention with hand-scheduled filler ========
            # qc0: remaining V inside head 0 (its ctx consumes v-tiles in
            # order); K dc1 + Q0 dc1 inside head 1 (head 2 = pt1 reads both);
            # Q1 in heads 2/3
            attention(0, CH, 2, [0, 1, 2, 3], {
                (0, 0): [U_v(0, 0)],
                (0, 1): [U_v(0, 1), U_v(0, 2)],
                (0, 2): [U_v(0, 3), U_v(1, 0)],
                (0, 3): [U_v(1, 1), U_v(1, 2)],
                (0, 4): [U_v(1, 3), U_v(2, 0)],
                (0, 5): [U_v(2, 1), U_v(2, 2)],
                (0, 6): [U_v(2, 3), U_v(3, 0)],
                (0, 7): [U_v(3, 1), U_v(3, 2), U_v(3, 3)],
                (1, 1): [U_qA(0, 1)], (1, 2): [U_qB(0, 1)],
                (1, 3): [U_kA(0)], (1, 4): [U_kB(0)],
                (1, 5): [U_kA(1)], (1, 6): [U_kB(1)],
                (1, 7): [U_kA(2)], (1, 8): [U_kB(2)],
                (2, 1): [U_kA(3)], (2, 2): [U_kB(3)],
                (2, 3): [U_qA(1, 0)], (2, 5): [U_qB(1, 0)],
                (3, 1): [U_qA(1, 1)], (3, 3): [U_qB(1, 1)],
            })
            # qc1: oproj(qc0) + Q2, one op-half + one parcel per head
            attention(CH, CH, 2, [0, 1, 2, 3], {
                (0, 2): [U_op(0, 0, 0)], (0, 4): [U_qA(2, 0)],
                (0, 6): [U_op(0, 0, 1)],
                (1, 2): [U_op(0, 1, 0)], (1, 4): [U_qB(2, 0)],
                (1, 6): [U_op(0, 1, 1)],
                (2, 2): [U_op(0, 2, 0)], (2, 4): [U_qA(2, 1)],
                (2, 6): [U_op(0, 2, 1)],
                (3, 2): [U_op(0, 3, 0)], (3, 4): [U_qB(2, 1)],
                (3, 6): [U_op(0, 3, 1)],
            })
            # qc2: oproj(qc1) + Q3 (two oproj halves deferred to qc3,
            # whose half-chunks otherwise run short of PE filler)
            attention(2 * CH, CH, 2, [0, 1, 2, 3], {
                (0, 1): [U_opA(1, 0, 0)], (0, 3): [U_opB(1, 0, 0)],
                (0, 4): [U_qA(3, 0)], (0, 6): [U_opA(1, 0, 1)],
                (1, 1): [U_opB(1, 0, 1)], (1, 3): [U_opA(1, 1, 0)],
                (1, 4): [U_qB(3, 0)], (1, 6): [U_opB(1, 1, 0)],
                (2, 1): [U_opA(1, 2, 0)], (2, 3): [U_opB(1, 2, 0)],
                (2, 4): [U_qA(3, 1)], (2, 6): [U_opA(1, 3, 0)],
                (3, 1): [U_opB(1, 3, 0)], (3, 3): [U_opA(1, 3, 1)],
                (3, 4): [U_qB(3, 1)], (3, 6): [U_opB(1, 3, 1)],
            })
            # qc3 first half-chunk: oproj(qc2) minus two halves that
            # rebalance into the second half
            attention(3 * CH, CH // 2, 4, [1, 3, 0, 2], {
                (0, 1): [U_op(1, 1, 1)], (0, 3): [U_op(2, 0, 0)],
                (1, 1): [U_op(1, 2, 1)], (1, 3): [U_op(2, 0, 1)],
                (2, 1): [U_op(2, 1, 0)], (2, 3): [U_op(2, 1, 1)],
                (3, 1): [U_op(2, 2, 0)], (3, 3): [U_op(2, 2, 1)],
            })
            # qc3 second half-chunk: leftover oproj(qc2) + oproj of the
            # first half (pieces 0,1)
            attention(3 * CH + CH // 2, CH // 2, 4, [1, 3, 0, 2], {
                (0, 1): [U_op(2, 3, 0)],
                (0, 3): [U_op(2, 3, 1)],
                (1, 1): [U_op(3, 0, 0)], (1, 3): [U_op(3, 0, 1)],
                (2, 1): [U_op(3, 1, 0)],
                (3, 1): [U_op(3, 1, 1)],
            }, pe_bcast_last=True)
            # final two pieces: the ctxT[0] halves of all four matmul
            # groups start during the last head's normalize chain (wfil
            # keeps the PE streak alive), the ctxT[1] halves land right
            # after it; DVE and ACT alternate the psum->sbuf copies and the
            # two out DMAs ride separate rings
            wfil = psp.tile([128, CH], F32, tag="qp", bufs=1, name="wfil")
            ob2 = wrk.tile([128, 2, D], BF16, tag="ob2")
            fin = []
            for wi in range(2):
                nc.tensor.matmul(wfil[:], lhsT=eye_sb[:],
                                 rhs=qrot[wi % 2][:, 0:CH],
                                 start=True, stop=True)
            for i, nt in enumerate((2, 3)):
                n0 = 3 * CH + nt * KT
                for oc in range(2):
                    psot = psp.tile([128, 2 * CH] if i == 0 else [128, CH],
                                    F32, tag="st" if i == 0 else "cx", bufs=2)
                    pso = psot[:, 0:CH]
                    nc.tensor.matmul(
                        pso, lhsT=ctxT[0][:, n0:n0 + KT],
                        rhs=wo_sb[:, 0, CH * oc:CH * (oc + 1)],
                        start=True, stop=False)
                    fin.append((pso, i, nt, oc))
            for wi in range(2):
                nc.tensor.matmul(wfil[:], lhsT=eye_sb[:],
                                 rhs=qrot[wi % 2][:, 0:CH],
                                 start=True, stop=True)
            for j, (pso, i, nt, oc) in enumerate(fin):
                n0 = 3 * CH + nt * KT
                nc.tensor.matmul(
                    pso, lhsT=ctxT[1][:, n0:n0 + KT],
                    rhs=wo_sb[:, 1, CH * oc:CH * (oc + 1)],
                    start=False, stop=True)
                if j % 2 == 0:
                    nc.vector.tensor_copy(
                        ob2[:, i, CH * oc:CH * (oc + 1)], pso)
                else:
                    nc.scalar.copy(
                        ob2[:, i, CH * oc:CH * (oc + 1)], pso)
                if oc == 1:
                    eng = nc.sync if i == 0 else nc.scalar
                    eng.dma_start(out[n0:n0 + KT, :], ob2[:, i, :])

    nc.compile()
    return nc


def _get_nc():
    if "nc" not in _CACHE:
        _CACHE["nc"] = _build()
    return _CACHE["nc"]


def _host_prep(x, rope_cos, rope_sin, Wq, bq, Wk, bk, Wv, bv, Wo, bo):
    perm64 = np.concatenate([np.arange(0, 64, 2), np.arange(1, 64, 2)])
    f = np.float32
    bf = ml_dtypes.bfloat16
    in_maps = []
    eyesw = np.zeros((128, 128), f)
    for c in range(128):
        eyesw[c, c ^ 32] = 1.0
    ones64_h = np.ones((1, 64), f)
    sign = np.tile(np.repeat(np.array([-1.0, 1.0], f), 32), C // 64)
    for core in range(DP * TP):
        b, r = divmod(core, TP)
        sel = np.concatenate([64 * (HPC * r + s) + perm64 for s in range(HPC)])
        xT = np.ascontiguousarray(x[b].T).astype(bf)
        cosT = np.ascontiguousarray(rope_cos[b][:, sel].T).astype(bf)
        sinT = (np.ascontiguousarray(rope_sin[b][:, sel].T)
                * sign[:, None]).astype(bf)
        wq_ = np.ascontiguousarray(Wq[sel, :].T).astype(bf)
        wk_ = np.ascontiguousarray(Wk[sel, :].T).astype(bf)
        wvx = np.zeros((D, HPC * VW), f)  # cast to bf16 below
        bvx = np.zeros((1, HPC * VW), f)
        for s in range(HPC):
            cols = sel[64 * s:64 * (s + 1)]
            wvx[:, VW * s:VW * s + HD] = Wv[cols, :].T
            bvx[0, VW * s:VW * s + HD] = bv[cols]
            bvx[0, VW * s + HD] = 1.0
        bvb = np.ascontiguousarray(np.broadcast_to(bvx, (128, HPC * VW)))
        bqk = np.stack([bq[sel].reshape(2, 128), bk[sel].reshape(2, 128)])
        woT = np.ascontiguousarray(Wo[:, sel].T)
        in_maps.append({
            "xT": xT, "cosT": cosT, "sinT": sinT, "ones64": ones64_h,
            "wq": wq_, "wk": wk_, "wvx": wvx.astype(bf), "bvb": bvb,
            "bqk": bqk.astype(f), "woT": woT, "eyesw": eyesw,
        })
    return in_maps


def kernel(x, rope_cos, rope_sin, Wq, bq, Wk, bk, Wv, bv, Wo, bo):
    nc = _get_nc()
    in_maps = _host_prep(np.asarray(x), np.asarray(rope_cos),
                         np.asarray(rope_sin), np.asarray(Wq), np.asarray(bq),
                         np.asarray(Wk), np.asarray(bk), np.asarray(Wv),
                         np.asarray(bv), np.asarray(Wo), np.asarray(bo))
    res = bass_utils.run_bass_kernel_spmd(
        nc, in_maps, core_ids=list(range(DP * TP)))
    out = np.zeros((B, N, D), np.float32)
    for core in range(DP * TP):
        b = core // TP
        out[b] += np.asarray(res.results[core]["out"], np.float32)
    out += np.asarray(bo)[None, None, :]
    return out


# revision 71
# speedup vs baseline: 1.0030x; 1.0030x over previous
"""Multi-head attention (B=2, N=2048, D=1024, 16 heads x 64) on 8 NeuronCores.

Sharding: data-parallel over batch (2) x tensor-parallel over heads (4 heads
per core). Each core computes q/k/v projections + RoPE + attention for its 4
heads and a partial output projection; the host sums the 4 tensor-parallel
partials per batch and adds the output bias.

Device kernel notes:
 - x / rope cos/sin are shipped and held in SBUF as bf16 (DMA is a single
   ~350 GB/s stream and otherwise gates the start); weights and all
   intermediates stay float32r (full-rate on the TRN2 PE).
 - One long PE stream: only the dc0 half of K (all chunks) plus the dc0
   half of Q chunk 0 are projected up front, then attention head 0 starts
   (~20us in) and every remaining projection / V-tile / output-projection
   piece is hand-placed as PE filler between attention score/ctx groups,
   sized so the exp (ACT) engine -- the attention-phase co-bottleneck --
   is never waited on.  In-attention projections are emitted as two
   4-matmul parcels accumulating in a dedicated PSUM bank so filler
   granularity matches the per-head ACT deficit.
 - RoPE is applied on channel-permuted q/k (evens-then-odds per 64-channel
   head block, folded into the weight slices host-side) so the rotate-pair
   step is a 32-partition-block swap: a permutation matmul pre-attention,
   and SBUF->SBUF DMAs (4 partition-block copies) for in-attention
   projections where PSUM banks and PE rows are the scarcer resource.
 - Scores are computed transposed (S^T[k, q]) so exp output feeds the ctx
   matmul directly; the softmax denominator comes from a ones-column
   appended to V; no per-row max -- a constant shift of 20 (free via the
   ACT bias, cancels in the ratio) keeps exp in fp32 range.
 - Softmax normalize: reciprocal on the denominator row (partition 64),
   a gpsimd cross-partition copy to partition 0 (HW partition_broadcast
   only reads partition 0), gpsimd broadcast, then one DVE multiply --
   no DMA in the chain.
 - The last q-chunk runs as two 256-column half-chunks (exp groups of 4
   k-tiles keep the ACT tiles 1024 wide) so the final oproj pieces of the
   first half overlap the second half and the drain is short.
"""
import sys

sys.path.insert(0, "/opt/trn_rl_repo")

import numpy as np
import ml_dtypes

import concourse.bacc as bacc
import concourse.mybir as mybir
import concourse.tile as tile
from concourse import bass_utils

B, N, D = 2, 2048, 1024
HEADS, HD = 16, 64
TP = 4                 # tensor-parallel ways (heads)
DP = 2                 # data-parallel ways (batch)
HPC = HEADS // TP      # heads per core = 4
C = HPC * HD           # channels per core = 256
CH = 512               # n-chunk size
NCH = N // CH          # 4
KT = 128               # k tile
NKT = N // KT          # 16
VW = HD + 1            # V columns per head incl. ones column = 65
ITC = D // KT          # 8 contraction tiles for projections
F32R = mybir.dt.float32r
F32 = mybir.dt.float32
BF16 = mybir.dt.bfloat16

_CACHE = {}


def _build():
    nc = bacc.Bacc("TRN2", debug=False, num_devices=DP * TP)

    xT = nc.dram_tensor("xT", [D, N], BF16, kind="ExternalInput").ap()
    cosT = nc.dram_tensor("cosT", [C, N], BF16, kind="ExternalInput").ap()
    sinT = nc.dram_tensor("sinT", [C, N], BF16, kind="ExternalInput").ap()
    wq = nc.dram_tensor("wq", [D, C], BF16, kind="ExternalInput").ap()
    wk = nc.dram_tensor("wk", [D, C], BF16, kind="ExternalInput").ap()
    wvx = nc.dram_tensor("wvx", [D, HPC * VW], BF16, kind="ExternalInput").ap()
    bvb = nc.dram_tensor("bvb", [128, HPC * VW], F32, kind="ExternalInput").ap()
    bqk = nc.dram_tensor("bqk", [2, 2, 128], F32, kind="ExternalInput").ap()
    woT = nc.dram_tensor("woT", [C, D], F32R, kind="ExternalInput").ap()
    eyesw = nc.dram_tensor("eyesw", [128, 128], F32R, kind="ExternalInput").ap()
    # partial sums only (host adds the 4 TP partials + bias) -- bf16 halves
    # the output DMA and shortens the drain
    out = nc.dram_tensor("out", [N, D], BF16, kind="ExternalOutput").ap()

    with tile.TileContext(nc) as tc:
        with tc.tile_pool(name="pers", bufs=1) as pers, \
             tc.tile_pool(name="wrk", bufs=1) as wrk, \
             tc.tile_pool(name="psp", bufs=1, space="PSUM") as psp:
            # warmup seed needs no DMA: PE can start its streak immediately
            wseed = pers.tile([128, 128], BF16, tag="wseed")
            nc.gpsimd.memset(wseed[:], 0.0)
            shift_sb = pers.tile([128, 1], F32, tag="shift")
            nc.gpsimd.memset(shift_sb[:], -20.0)
            # tiny dummy exp: pulls the ACT exp-table load (1.3us) to t~1us
            dume = wrk.tile([1, 8], F32, tag="dume")
            nc.gpsimd.memset(dume[:], 0.5)
            dume2 = wrk.tile([1, 8], F32, tag="dume2")
            nc.scalar.activation(dume2[:], dume[:],
                                 mybir.ActivationFunctionType.Exp)

            # ---- persistent SBUF; DMA emission order = arrival order on the
            # single ~350GB/s stream, so it is chosen to match first use ----
            eye_sb = pers.tile([128, 128], F32R, tag="eyesw")
            nc.sync.dma_start(eye_sb[:], eyesw)
            wk_sb = pers.tile([128, ITC, C], BF16, tag="wk")
            wq_sb = pers.tile([128, ITC, C], BF16, tag="wq")
            xt_sb = pers.tile([128, NCH, ITC, CH], BF16, tag="xt")
            cos_sb = pers.tile([128, 2, NCH, CH], BF16, tag="cos")
            sin_sb = pers.tile([128, 2, NCH, CH], BF16, tag="sin")
            wv_sb = pers.tile([128, ITC, HPC * VW], BF16, tag="wv")
            bvb_sb = pers.tile([128, HPC * VW], F32, tag="bvb")
            bqk_sb = pers.tile([128, 2, 2], F32, tag="bqk")
            wo_sb = pers.tile([128, 2, D], F32R, tag="wo")

            def load_w_dc(w_sb, w, dc):
                nc.sync.dma_start(
                    w_sb[:, :, 128 * dc:128 * (dc + 1)],
                    w[:, 128 * dc:128 * (dc + 1)].rearrange(
                        "(t p) c -> p t c", p=128))

            def load_x(c):
                half = ITC // 2
                nc.sync.dma_start(
                    xt_sb[:, c, :half, :],
                    xT[:half * 128, c * CH:(c + 1) * CH].rearrange(
                        "(t p) n -> p t n", p=128))
                nc.sync.dma_start(
                    xt_sb[:, c, half:, :],
                    xT[half * 128:, c * CH:(c + 1) * CH].rearrange(
                        "(t p) n -> p t n", p=128))

            def load_cs(c, t):
                nc.sync.dma_start(
                    cos_sb[:, t, c, :],
                    cosT[128 * t:128 * (t + 1), c * CH:(c + 1) * CH])
                nc.sync.dma_start(
                    sin_sb[:, t, c, :],
                    sinT[128 * t:128 * (t + 1), c * CH:(c + 1) * CH])

            load_w_dc(wk_sb, wk, 0)
            load_x(0)
            load_cs(0, 0)
            nc.sync.dma_start(bqk_sb[:], bqk.rearrange("a c p -> p a c"))
            load_x(1)
            load_cs(1, 0)
            load_w_dc(wq_sb, wq, 0)
            load_x(2)
            load_cs(2, 0)
            load_x(3)
            load_cs(3, 0)
            nc.sync.dma_start(
                wv_sb[:], wvx.rearrange("(t p) c -> p t c", p=128))
            nc.sync.dma_start(bvb_sb[:], bvb)
            load_w_dc(wk_sb, wk, 1)
            load_w_dc(wq_sb, wq, 1)
            for c in range(NCH):
                load_cs(c, 1)
            nc.sync.dma_start(wo_sb[:], woT.rearrange("(t p) o -> p t o", p=128))

            # p-state warmup: keep the PE streak alive through the initial
            # DMA wait so the first projections run at full clock
            wps = psp.tile([128, 128], F32, tag="aux", bufs=1, name="warmup")
            for wi in range(20):
                nc.tensor.matmul(wps[:], lhsT=wseed[:], rhs=wseed[:],
                                 start=True, stop=True)
            for wi in range(18):
                nc.tensor.matmul(wps[:], lhsT=eye_sb[:], rhs=eye_sb[:],
                                 start=True, stop=True)
            qrot = [pers.tile([128, N], F32R, tag=f"qrot{t}", name=f"qrot{t}")
                    for t in range(2)]
            krot = [pers.tile([128, N], F32R, tag=f"krot{t}", name=f"krot{t}")
                    for t in range(2)]
            v_sb = [pers.tile([128, HPC * VW], F32R, tag=f"v{t}", name=f"v{t}")
                    for t in range(NKT)]
            ctxT = [pers.tile([128, N], F32R, tag=f"ctxT{t}", name=f"ctxT{t}")
                    for t in range(2)]

            # pre-attention projections in two stages so the rotate eye
            # matmul (gated by a DVE add) never head-of-line blocks the PE
            _pa = {}

            def rope_a_mm(w_sb, qk, key, c, dc):
                # scores psum (tag "st") is idle pre-attention -- use its
                # two banks as the proj/rotate pair
                stt = psp.tile([128, 2 * CH], F32, tag="st", bufs=2)
                ps, pssh = stt[:, 0:CH], stt[:, CH:2 * CH]
                for it in range(ITC):
                    nc.tensor.matmul(
                        ps, lhsT=w_sb[:, it, 128 * dc:128 * (dc + 1)],
                        rhs=xt_sb[:, c, it, :],
                        start=(it == 0), stop=(it == ITC - 1))
                raw = wrk.tile([128, CH], F32R, tag="raw", bufs=4)
                nc.vector.tensor_scalar_add(
                    raw[:], ps, bqk_sb[:, qk, dc:dc + 1])
                _pa[key] = (raw, pssh)

            def rope_a_fin(key, dst, c, dc):
                raw, pssh = _pa.pop(key)
                nc.tensor.matmul(pssh, lhsT=eye_sb[:], rhs=raw[:],
                                 start=True, stop=True)
                m1 = wrk.tile([128, CH], F32, tag="m1", bufs=2)
                nc.vector.tensor_mul(m1[:], raw[:], cos_sb[:, dc, c, :])
                m2 = wrk.tile([128, CH], F32, tag="m2", bufs=2)
                nc.vector.tensor_mul(m2[:], pssh, sin_sb[:, dc, c, :])
                nc.vector.tensor_add(
                    dst[dc][:, c * CH:(c + 1) * CH], m1[:], m2[:])

            # in-attention projections: two 4-matmul parcels into the
            # dedicated "qp" bank; rotate-swap via 4 SBUF->SBUF DMAs
            _pb = {}

            def proj_b_A(w_sb, dc, c):
                ps = psp.tile([128, CH], F32, tag="qp", bufs=1)
                for it in range(4):
                    nc.tensor.matmul(
                        ps[:], lhsT=w_sb[:, it, 128 * dc:128 * (dc + 1)],
                        rhs=xt_sb[:, c, it, :],
                        start=(it == 0), stop=False)
                _pb["ps"] = ps

            def proj_b_B(w_sb, qk, dst, c, dc):
                ps = _pb.pop("ps")
                for it in range(4, ITC):
                    nc.tensor.matmul(
                        ps[:], lhsT=w_sb[:, it, 128 * dc:128 * (dc + 1)],
                        rhs=xt_sb[:, c, it, :],
                        start=False, stop=(it == ITC - 1))
                raw = wrk.tile([128, CH], F32R, tag="raw", bufs=4)
                nc.vector.tensor_scalar_add(
                    raw[:], ps[:], bqk_sb[:, qk, dc:dc + 1])
                rswp = wrk.tile([128, CH], F32R, tag="rswp", bufs=2)
                for b in range(4):
                    nc.sync.dma_start(rswp[32 * b:32 * (b + 1), :],
                                      raw[32 * (b ^ 1):32 * ((b ^ 1) + 1), :])
                m1 = wrk.tile([128, CH], F32, tag="m1", bufs=2)
                nc.vector.tensor_mul(m1[:], raw[:], cos_sb[:, dc, c, :])
                m2 = wrk.tile([128, CH], F32, tag="m2", bufs=2)
                nc.vector.tensor_mul(m2[:], rswp[:], sin_sb[:, dc, c, :])
                nc.vector.tensor_add(
                    dst[dc][:, c * CH:(c + 1) * CH], m1[:], m2[:])

            def proj_b_B_eye(w_sb, qk, dst, c, dc):
                ps = _pb.pop("ps")
                for it in range(4, ITC):
                    nc.tensor.matmul(
                        ps[:], lhsT=w_sb[:, it, 128 * dc:128 * (dc + 1)],
                        rhs=xt_sb[:, c, it, :],
                        start=False, stop=(it == ITC - 1))
                raw = wrk.tile([128, CH], F32R, tag="raw", bufs=4)
                nc.vector.tensor_scalar_add(
                    raw[:], ps[:], bqk_sb[:, qk, dc:dc + 1])
                pssh = psp.tile([128, CH], F32, tag="aux", bufs=1)
                nc.tensor.matmul(pssh[:], lhsT=eye_sb[:], rhs=raw[:],
                                 start=True, stop=True)
                m1 = wrk.tile([128, CH], F32, tag="m1", bufs=2)
                nc.vector.tensor_mul(m1[:], raw[:], cos_sb[:, dc, c, :])
                m2 = wrk.tile([128, CH], F32, tag="m2", bufs=2)
                nc.vector.tensor_mul(m2[:], pssh[:], sin_sb[:, dc, c, :])
                nc.vector.tensor_add(
                    dst[dc][:, c * CH:(c + 1) * CH], m1[:], m2[:])

            def vproj_tile(c, vt):
                kt = c * (CH // KT) + vt
                psv = psp.tile([128, CH], F32, tag="aux", bufs=1)
                for it in range(ITC):
                    nc.tensor.matmul(
                        psv[:, 0:HPC * VW],
                        lhsT=xt_sb[:, c, it, KT * vt:KT * (vt + 1)],
                        rhs=wv_sb[:, it, :],
                        start=(it == 0), stop=(it == ITC - 1))
                nc.vector.tensor_add(v_sb[kt][:], psv[:, 0:HPC * VW], bvb_sb[:])

            # one bf16 staging tile per KT-row piece; both oc halves copy in,
            # the second flushes with a single DMA
            _obs = {}

            def oproj_half(qc8, nt, oc, pstag="aux"):
                n0 = qc8 * CH + nt * KT
                pso = psp.tile([128, 2 * CH] if pstag == "st" else [128, CH],
                               F32, tag=pstag, bufs=2 if pstag == "st" else 1)
                for it in range(2):
                    nc.tensor.matmul(
                        pso[:, 0:CH], lhsT=ctxT[it][:, n0:n0 + KT],
                        rhs=wo_sb[:, it, CH * oc:CH * (oc + 1)],
                        start=(it == 0), stop=(it == 1))
                if oc == 0:
                    ob = wrk.tile([128, D], BF16, tag="ob", bufs=4)
                    _obs[(qc8, nt)] = ob
                else:
                    ob = _obs.pop((qc8, nt))
                nc.vector.tensor_copy(ob[:, CH * oc:CH * (oc + 1)],
                                      pso[:, 0:CH])
                if oc == 1:
                    nc.sync.dma_start(out[n0:n0 + KT, :], ob[:])

            # ---- filler units ----
            def U_kA(c):
                return lambda: proj_b_A(wk_sb, 1, c)

            def U_kB(c):
                return lambda: proj_b_B(wk_sb, 1, krot, c, 1)

            def U_qA(c, dc):
                return lambda: proj_b_A(wq_sb, dc, c)

            def U_qB(c, dc):
                return lambda: proj_b_B(wq_sb, 0, qrot, c, dc)

            def U_k0A(c):
                return lambda: proj_b_A(wk_sb, 0, c)

            def U_k0B(c):
                return lambda: proj_b_B_eye(wk_sb, 1, krot, c, 0)

            def U_v(c, vt):
                return lambda: vproj_tile(c, vt)

            def U_op(qc8, nt, oc):
                return lambda: oproj_half(qc8, nt, oc)

            _opq = {}

            def U_opA(qc8, nt, oc):
                def f():
                    n0 = qc8 * CH + nt * KT
                    pso = psp.tile([128, CH], F32, tag="aux", bufs=1,
                                   name=f"opq{qc8}_{nt}_{oc}")
                    nc.tensor.matmul(
                        pso[:], lhsT=ctxT[0][:, n0:n0 + KT],
                        rhs=wo_sb[:, 0, CH * oc:CH * (oc + 1)],
                        start=True, stop=False)
                    _opq[(qc8, nt, oc)] = pso
                return f

            def U_opB(qc8, nt, oc):
                def f():
                    n0 = qc8 * CH + nt * KT
                    pso = _opq.pop((qc8, nt, oc))
                    nc.tensor.matmul(
                        pso[:], lhsT=ctxT[1][:, n0:n0 + KT],
                        rhs=wo_sb[:, 1, CH * oc:CH * (oc + 1)],
                        start=False, stop=True)
                    if oc == 0:
                        ob = wrk.tile([128, D], BF16, tag="ob", bufs=4)
                        _obs[(qc8, nt)] = ob
                    else:
                        ob = _obs.pop((qc8, nt))
                    nc.vector.tensor_copy(ob[:, CH * oc:CH * (oc + 1)],
                                          pso[:])
                    if oc == 1:
                        nc.sync.dma_start(out[n0:n0 + KT, :], ob[:])
                return f

            def attention(q0, qw, gk, horder, fill, pe_bcast_last=False):
                """Attention for q columns [q0, q0+qw); gk k-tiles per exp
                group. fill: {(hi, g): [units]} emitted after scores(g);
                g == ngr means after the head's last ctx."""
                ngr = NKT // gk
                qs = slice(q0, q0 + qw)
                for hi, h in enumerate(horder):
                    pt, par = h // 2, h % 2
                    cx = psp.tile([128, CH], F32, tag="cx", bufs=2)
                    r0 = 64 * par
                    es_q = [None] * ngr

                    def scores(g):
                        stg = psp.tile([128, 2 * CH], F32, tag="st",
                                       bufs=2, name=f"st{q0}_{h}_{g}")
                        for j in range(gk):
                            kt = gk * g + j
                            nc.tensor.matmul(
                                stg[:, qw * j:qw * (j + 1)],
                                lhsT=krot[pt][r0:r0 + 64, KT * kt:KT * (kt + 1)],
                                rhs=qrot[pt][r0:r0 + 64, qs],
                                start=True, stop=True)
                        es = wrk.tile([128, 2 * CH], F32R, tag="es",
                                      bufs=4, name=f"es{q0}_{h}_{g}")
                        # constant shift cancels in the softmax ratio but
                        # widens the no-max-subtraction overflow envelope
                        nc.scalar.activation(
                            es[:, 0:gk * qw], stg[:, 0:gk * qw],
                            mybir.ActivationFunctionType.Exp,
                            bias=shift_sb[:])
                        es_q[g] = es

                    def ctx_acc(g):
                        for j in range(gk):
                            kt = gk * g + j
                            nc.tensor.matmul(
                                cx[0:VW, 0:qw],
                                lhsT=v_sb[kt][:, VW * h:VW * (h + 1)],
                                rhs=es_q[g][:, qw * j:qw * (j + 1)],
                                start=(kt == 0), stop=(kt == NKT - 1))

                    def emit_fill(g):
                        for u in fill.get((hi, g), ()):
                            u()

                    scores(0)
                    emit_fill(0)
                    for g in range(1, ngr):
                        scores(g)
                        emit_fill(g)
                        ctx_acc(g - 1)
                    ctx_acc(ngr - 1)
                    emit_fill(ngr)

                    rt = wrk.tile([128, CH], F32, tag="rt", bufs=2)
                    nc.vector.reciprocal(rt[HD:HD + 1, 0:qw],
                                         cx[HD:HD + 1, 0:qw])
                    # hop the denominator row to partition 0 with a
                    # gpsimd copy (HW partition_broadcast only reads
                    # partition 0; cross-partition gpsimd access works for
                    # 32-aligned partitions), then broadcast -- no DMA
                    rt0 = wrk.tile([1, CH], F32, tag="rt0", bufs=2)
                    nc.gpsimd.tensor_copy(rt0[:, 0:qw], rt[HD:HD + 1, 0:qw])
                    rb = wrk.tile([64, CH], F32, tag="rb", bufs=2)
                    nc.gpsimd.partition_broadcast(rb[:, 0:qw],
                                                  rt0[:, 0:qw])
                    rbv = rb
                    if par == 0:
                        if pe_bcast_last and hi == len(horder) - 1:
                            # split by column half: each final oproj piece
                            # unblocks as soon as its columns are scaled
                            hw_ = qw // 2
                            nc.vector.tensor_mul(
                                ctxT[pt][0:64, q0:q0 + hw_],
                                cx[0:64, 0:hw_], rbv[:, 0:hw_])
                            nc.vector.tensor_mul(
                                ctxT[pt][0:64, q0 + hw_:q0 + qw],
                                cx[0:64, hw_:qw], rbv[:, hw_:qw])
                        else:
                            nc.vector.tensor_mul(
                                ctxT[pt][0:64, qs], cx[0:64, 0:qw],
                                rbv[:, 0:qw])
                    else:
                        ch_t = wrk.tile([64, CH], F32R, tag="ch", bufs=2)
                        nc.vector.tensor_mul(ch_t[:, 0:qw], cx[0:64, 0:qw],
                                             rbv[:, 0:qw])
                        nc.sync.dma_start(ctxT[pt][64:128, qs], ch_t[:, 0:qw])

            # ======== pre-attention: pure K dc0 + Q0 dc0 chain -- x3 is
            # the critical-path entry, so V0/V1 ride inside attention head 0
            # and all x chunks precede wv on the DMA stream
            rope_a_mm(wk_sb, 1, "k0", 0, 0)
            rope_a_mm(wk_sb, 1, "k1", 1, 0)
            rope_a_fin("k0", krot, 0, 0)
            rope_a_mm(wq_sb, 0, "q0", 0, 0)
            rope_a_fin("k1", krot, 1, 0)
            rope_a_mm(wk_sb, 1, "k2", 2, 0)
            rope_a_fin("q0", qrot, 0, 0)
            for wi in range(8):
                nc.tensor.matmul(wps[:], lhsT=eye_sb[:], rhs=eye_sb[:],
                                 start=True, stop=True)
            rope_a_mm(wk_sb, 1, "k3", 3, 0)
            rope_a_fin("k2", krot, 2, 0)
            rope_a_fin("k3", krot, 3, 0)

            # ======== attention with hand-scheduled filler ========
            # qc0: remaining V inside head 0 (its ctx consumes v-tiles in
            # order); K dc1 + Q0 dc1 inside head 1 (head 2 = pt1 reads both);
            # Q1 in heads 2/3
            attention(0, CH, 2, [0, 1, 2, 3], {
                (0, 0): [U_v(0, 0)],
                (0, 1): [U_v(0, 1), U_v(0, 2)],
                (0, 2): [U_v(0, 3), U_v(1, 0)],
                (0, 3): [U_v(1, 1), U_v(1, 2)],
                (0, 4): [U_v(1, 3), U_v(2, 0)],
                (0, 5): [U_v(2, 1), U_v(2, 2)],
                (0, 6): [U_v(2, 3), U_v(3, 0)],
                (0, 7): [U_v(3, 1), U_v(3, 2), U_v(3, 3)],
                (1, 1): [U_qA(0, 1)], (1, 2): [U_qB(0, 1)],
                (1, 3): [U_kA(0)], (1, 4): [U_kB(0)],
                (1, 5): [U_kA(1)], (1, 6): [U_kB(1)],
                (1, 7): [U_kA(2)], (1, 8): [U_kB(2)],
                (2, 1): [U_kA(3)], (2, 2): [U_kB(3)],
                (2, 3): [U_qA(1, 0)], (2, 5): [U_qB(1, 0)],
                (3, 1): [U_qA(1, 1)], (3, 3): [U_qB(1, 1)],
            })
            # qc1: oproj(qc0) + Q2, one op-half + one parcel per head
            attention(CH, CH, 2, [0, 1, 2, 3], {
                (0, 2): [U_op(0, 0, 0)], (0, 4): [U_qA(2, 0)],
                (0, 6): [U_op(0, 0, 1)],
                (1, 2): [U_op(0, 1, 0)], (1, 4): [U_qB(2, 0)],
                (1, 6): [U_op(0, 1, 1)],
                (2, 2): [U_op(0, 2, 0)], (2, 4): [U_qA(2, 1)],
                (2, 6): [U_op(0, 2, 1)],
                (3, 2): [U_op(0, 3, 0)], (3, 4): [U_qB(2, 1)],
                (3, 6): [U_op(0, 3, 1)],
            })
            # qc2: oproj(qc1) + Q3 (two oproj halves deferred to qc3,
            # whose half-chunks otherwise run short of PE filler)
            attention(2 * CH, CH, 2, [0, 1, 2, 3], {
                (0, 1): [U_opA(1, 0, 0)], (0, 3): [U_opB(1, 0, 0)],
                (0, 4): [U_qA(3, 0)], (0, 6): [U_opA(1, 0, 1)],
                (1, 1): [U_opB(1, 0, 1)], (1, 3): [U_opA(1, 1, 0)],
                (1, 4): [U_qB(3, 0)], (1, 6): [U_opB(1, 1, 0)],
                (2, 1): [U_opA(1, 2, 0)], (2, 3): [U_opB(1, 2, 0)],
                (2, 4): [U_qA(3, 1)], (2, 6): [U_opA(1, 3, 0)],
                (3, 1): [U_opB(1, 3, 0)], (3, 3): [U_opA(1, 3, 1)],
                (3, 4): [U_qB(3, 1)], (3, 6): [U_opB(1, 3, 1)],
            })
            # qc3 first half-chunk: oproj(qc2) minus two halves that
            # rebalance into the second half
            attention(3 * CH, CH // 2, 4, [1, 3, 0, 2], {
                (0, 1): [U_op(1, 1, 1)], (0, 3): [U_op(2, 0, 0)],
                (1, 1): [U_op(1, 2, 1)], (1, 3): [U_op(2, 0, 1)],
                (2, 1): [U_op(2, 1, 0)], (2, 3): [U_op(2, 1, 1)],
                (3, 1): [U_op(2, 2, 0)], (3, 3): [U_op(2, 2, 1)],
            })
            # qc3 second half-chunk: leftover oproj(qc2) + oproj of the
            # first half (pieces 0,1)
            attention(3 * CH + CH // 2, CH // 2, 4, [1, 3, 0, 2], {
                (0, 1): [U_op(2, 3, 0)],
                (0, 3): [U_op(2, 3, 1)],
                (1, 1): [U_op(3, 0, 0)], (1, 3): [U_op(3, 0, 1)],
                (2, 1): [U_op(3, 1, 0)],
                (3, 1): [U_op(3, 1, 1)],
            }, pe_bcast_last=True)
            # final two pieces: the ctxT[0] halves of all four matmul
            # groups start during the last head's normalize chain (wfil
            # keeps the PE streak alive), the ctxT[1] halves land right
            # after it; DVE and ACT alternate the psum->sbuf copies and the
            # two out DMAs ride separate rings
            wfil = psp.tile([128, CH], F32, tag="qp", bufs=1, name="wfil")
            ob2 = wrk.tile([128, 2, D], BF16, tag="ob2")
            fin = []
            for wi in range(2):
                nc.tensor.matmul(wfil[:], lhsT=eye_sb[:],
                                 rhs=qrot[wi % 2][:, 0:CH],
                                 start=True, stop=True)
            for i, nt in enumerate((2, 3)):
                n0 = 3 * CH + nt * KT
                for oc in range(2):
                    psot = psp.tile([128, 2 * CH] if i == 0 else [128, CH],
                                    F32, tag="st" if i == 0 else "cx", bufs=2)
                    pso = psot[:, 0:CH]
                    nc.tensor.matmul(
                        pso, lhsT=ctxT[0][:, n0:n0 + KT],
                        rhs=wo_sb[:, 0, CH * oc:CH * (oc + 1)],
                        start=True, stop=False)
                    fin.append((pso, i, nt, oc))
            for wi in range(2):
                nc.tensor.matmul(wfil[:], lhsT=eye_sb[:],
                                 rhs=qrot[wi % 2][:, 0:CH],
                                 start=True, stop=True)
            for j, (pso, i, nt, oc) in enumerate(fin):
                n0 = 3 * CH + nt * KT
                nc.tensor.matmul(
                    pso, lhsT=ctxT[1][:, n0:n0 + KT],
                    rhs=wo_sb[:, 1, CH * oc:CH * (oc + 1)],
                    start=False, stop=True)
                if j % 2 == 0:
                    nc.vector.tensor_copy(
                        ob2[:, i, CH * oc:CH * (oc + 1)], pso)
                else:
                    nc.scalar.copy(
                        ob2[:, i, CH * oc:CH * (oc + 1)], pso)
                if oc == 1:
                    eng = nc.sync if i == 0 else nc.scalar
                    eng.dma_start(out[n0:n0 + KT, :], ob2[:, i, :])

    nc.compile()
    return nc


def _get_nc():
    if "nc" not in _CACHE:
        _CACHE["nc"] = _build()
    return _CACHE["nc"]


def _host_prep(x, rope_cos, rope_sin, Wq, bq, Wk, bk, Wv, bv, Wo, bo):
    perm64 = np.concatenate([np.arange(0, 64, 2), np.arange(1, 64, 2)])
    f = np.float32
    bf = ml_dtypes.bfloat16
    in_maps = []
    eyesw = np.zeros((128, 128), f)
    for c in range(128):
        eyesw[c, c ^ 32] = 1.0
    sign = np.tile(np.repeat(np.array([-1.0, 1.0], f), 32), C // 64)
    for core in range(DP * TP):
        b, r = divmod(core, TP)
        sel = np.concatenate([64 * (HPC * r + s) + perm64 for s in range(HPC)])
        xT = np.ascontiguousarray(x[b].T).astype(bf)
        cosT = np.ascontiguousarray(rope_cos[b][:, sel].T).astype(bf)
        sinT = (np.ascontiguousarray(rope_sin[b][:, sel].T)
                * sign[:, None]).astype(bf)
        wq_ = np.ascontiguousarray(Wq[sel, :].T).astype(bf)
        wk_ = np.ascontiguousarray(Wk[sel, :].T).astype(bf)
        wvx = np.zeros((D, HPC * VW), f)  # cast to bf16 below
        bvx = np.zeros((1, HPC * VW), f)
        for s in range(HPC):
            cols = sel[64 * s:64 * (s + 1)]
            wvx[:, VW * s:VW * s + HD] = Wv[cols, :].T
            bvx[0, VW * s:VW * s + HD] = bv[cols]
            bvx[0, VW * s + HD] = 1.0
        bvb = np.ascontiguousarray(np.broadcast_to(bvx, (128, HPC * VW)))
        bqk = np.stack([bq[sel].reshape(2, 128), bk[sel].reshape(2, 128)])
        woT = np.ascontiguousarray(Wo[:, sel].T)
        in_maps.append({
            "xT": xT, "cosT": cosT, "sinT": sinT,
            "wq": wq_, "wk": wk_, "wvx": wvx.astype(bf), "bvb": bvb,
            "bqk": bqk.astype(f), "woT": woT, "eyesw": eyesw,
        })
    return in_maps


def kernel(x, rope_cos, rope_sin, Wq, bq, Wk, bk, Wv, bv, Wo, bo):
    nc = _get_nc()
    in_maps = _host_prep(np.asarray(x), np.asarray(rope_cos),
                         np.asarray(rope_sin), np.asarray(Wq), np.asarray(bq),
                         np.asarray(Wk), np.asarray(bk), np.asarray(Wv),
                         np.asarray(bv), np.asarray(Wo), np.asarray(bo))
    res = bass_utils.run_bass_kernel_spmd(
        nc, in_maps, core_ids=list(range(DP * TP)))
    out = np.zeros((B, N, D), np.float32)
    for core in range(DP * TP):
        b = core // TP
        out[b] += np.asarray(res.results[core]["out"], np.float32)
    out += np.asarray(bo)[None, None, :]
    return out
